# revision 1
# baseline (speedup 1.0000x reference)
"""Expert-parallel MoE MLP (Llama4 text experts) for 8 Trainium2 NeuronCores.

Strategy: core e handles expert e. Tokens are grouped by expert on the host
(indices are sorted, but we argsort for robustness), padded to a common
T_pad, and each core runs a dense gated-MLP over its token block:
    y = (up * silu(gate)) @ W_d,   [gate | up] = x @ W_gu
Everything is computed in the transposed layout (weights are the stationary
matmul operand, tokens stream):  y^T = W_d^T @ (h^T),  h^T = up^T * silu(W_gu^T @ x^T).
bf16 inputs, fp32 PSUM accumulation, fp32 output.
"""

import numpy as np
import ml_dtypes

_BF16 = ml_dtypes.bfloat16
_NC = 8  # cores

_nc_cache: dict = {}
last_run = None  # BassKernelResults of the most recent kernel() call (for test harness)


def _build(T_pad: int, H: int, F: int):
    import concourse.bacc as bacc
    import concourse.mybir as mybir
    from concourse.tile import TileContext

    nc = bacc.Bacc()
    F2 = 2 * F
    xT = nc.dram_tensor("xT", [H, T_pad], mybir.dt.bfloat16, kind="ExternalInput")
    wgu = nc.dram_tensor("wgu", [H, F2], mybir.dt.bfloat16, kind="ExternalInput")
    wd = nc.dram_tensor("wd", [F, H], mybir.dt.bfloat16, kind="ExternalInput")
    yT = nc.dram_tensor("yT", [H, T_pad], mybir.dt.float32, kind="ExternalOutput")

    KB1 = H // 128   # contraction chunks for x @ W_gu
    NF = F2 // 128   # 2F output tiles (first half gate, second half up)
    NG = NF // 2
    KB2 = F // 128   # contraction chunks for h @ W_d
    NH = H // 128    # output tiles of y

    blocks = []
    t0 = 0
    while t0 < T_pad:
        nb = min(512, T_pad - t0)
        blocks.append((t0, nb))
        t0 += nb

    with TileContext(nc) as tc:
        with (
            tc.tile_pool(name="wgu_p", bufs=1) as wgu_p,
            tc.tile_pool(name="wd_p", bufs=1) as wd_p,
            tc.tile_pool(name="x_p", bufs=1) as x_p,
            tc.tile_pool(name="silu_p", bufs=3) as silu_p,
            tc.tile_pool(name="up_p", bufs=3) as up_p,
            tc.tile_pool(name="h_p", bufs=2) as h_p,
            tc.tile_pool(name="y_p", bufs=2) as y_p,
            tc.tile_pool(name="ps1", bufs=4, space="PSUM") as ps1_p,
            tc.tile_pool(name="ps2", bufs=3, space="PSUM") as ps2_p,
        ):
            # x^T first: every matmul needs it.
            x_sb = []
            for k in range(KB1):
                t = x_p.tile([128, T_pad], mybir.dt.bfloat16, name=f"x{k}", tag=f"x{k}")
                x_sb.append(t)
            for t0, nb in blocks:
                for k in range(KB1):
                    nc.sync.dma_start(
                        out=x_sb[k][:, t0 : t0 + nb],
                        in_=xT[k * 128 : (k + 1) * 128, t0 : t0 + nb],
                    )
            # W_gu: allocate whole-row-chunk tiles but DMA by 512-col groups,
            # f-group-major, so the first f-tiles' weights land first.
            wgu_sb = [
                wgu_p.tile([128, F2], mybir.dt.bfloat16, name=f"wgu{k}", tag=f"wgu{k}")
                for k in range(KB1)
            ]
            ngrp = F2 // 512
            # gate group g and up group g+ngrp/2 are consumed together — interleave
            order = [g for pair in zip(range(ngrp // 2), range(ngrp // 2, ngrp)) for g in pair]
            for g in order:
                cs = slice(g * 512, (g + 1) * 512)
                for k in range(KB1):
                    nc.sync.dma_start(
                        out=wgu_sb[k][:, cs], in_=wgu[k * 128 : (k + 1) * 128, cs]
                    )
            wd_sb = []
            for k in range(KB2):
                t = wd_p.tile([128, H], mybir.dt.bfloat16, name=f"wd{k}", tag=f"wd{k}")
                nc.sync.dma_start(out=t, in_=wd[k * 128 : (k + 1) * 128, :])
                wd_sb.append(t)

            for t0, nb in blocks:
                ts = slice(t0, t0 + nb)
                h_tiles = []
                # gate tile i and up tile i+NG paired so the silu tile dies fast
                for i in range(NG):
                    ps_g = ps1_p.tile([128, 512], mybir.dt.float32, tag="ps1")
                    for k in range(KB1):
                        nc.tensor.matmul(
                            out=ps_g[:, :nb],
                            lhsT=wgu_sb[k][:, i * 128 : (i + 1) * 128],
                            rhs=x_sb[k][:, ts],
                            start=(k == 0),
                            stop=(k == KB1 - 1),
                        )
                    st = silu_p.tile([128, 512], mybir.dt.bfloat16, tag="silu")
                    nc.scalar.activation(
                        st[:, :nb], ps_g[:, :nb], mybir.ActivationFunctionType.Silu
                    )
                    ps_u = ps1_p.tile([128, 512], mybir.dt.float32, tag="ps1")
                    iu = i + NG
                    for k in range(KB1):
                        nc.tensor.matmul(
                            out=ps_u[:, :nb],
                            lhsT=wgu_sb[k][:, iu * 128 : (iu + 1) * 128],
                            rhs=x_sb[k][:, ts],
                            start=(k == 0),
                            stop=(k == KB1 - 1),
                        )
                    ut = up_p.tile([128, 512], mybir.dt.bfloat16, tag="up")
                    nc.vector.tensor_copy(ut[:, :nb], ps_u[:, :nb])
                    ht = h_p.tile([128, 512], mybir.dt.bfloat16, tag=f"h{i}")
                    nc.vector.tensor_mul(
                        out=ht[:, :nb], in0=ut[:, :nb], in1=st[:, :nb]
                    )
                    h_tiles.append(ht)
                for hh in range(NH):
                    ps_y = ps2_p.tile([128, 512], mybir.dt.float32, tag="ps2")
                    for k in range(KB2):
                        nc.tensor.matmul(
                            out=ps_y[:, :nb],
                            lhsT=wd_sb[k][:, hh * 128 : (hh + 1) * 128],
                            rhs=h_tiles[k][:, :nb],
                            start=(k == 0),
                            stop=(k == KB2 - 1),
                        )
                    yt = y_p.tile([128, 512], mybir.dt.float32, tag=f"y{hh}")
                    nc.vector.tensor_copy(yt[:, :nb], ps_y[:, :nb])
                    nc.sync.dma_start(
                        out=yT[hh * 128 : (hh + 1) * 128, ts], in_=yt[:, :nb]
                    )
    nc.compile()
    return nc


def kernel(hidden_states, local_expert_indices, gate_up_proj, down_proj):
    from concourse.bass_utils import run_bass_kernel_spmd

    x = np.asarray(hidden_states, dtype=np.float32)
    idx = np.asarray(local_expert_indices).astype(np.int64)
    wgu_all = np.asarray(gate_up_proj, dtype=np.float32)
    wd_all = np.asarray(down_proj, dtype=np.float32)

    T, H = x.shape
    E, _, F2 = wgu_all.shape
    F = F2 // 2
    assert E == _NC

    order = np.argsort(idx, kind="stable")
    counts = np.bincount(idx, minlength=E)
    starts = np.concatenate([[0], np.cumsum(counts)])
    T_pad = max(512, int(-(-counts.max() // 128) * 128))

    key = (T_pad, H, F)
    if key not in _nc_cache:
        _nc_cache[key] = _build(T_pad, H, F)
    nc = _nc_cache[key]

    x_sorted = x[order]
    in_maps = []
    for e in range(E):
        s, c = int(starts[e]), int(counts[e])
        xb = np.zeros((T_pad, H), np.float32)
        xb[:c] = x_sorted[s : s + c]
        in_maps.append(
            {
                "xT": np.ascontiguousarray(xb.T).astype(_BF16),
                "wgu": wgu_all[e].astype(_BF16),
                "wd": wd_all[e].astype(_BF16),
            }
        )

    res = run_bass_kernel_spmd(nc, in_maps, core_ids=list(range(_NC)))
    global last_run
    last_run = res

    out = np.zeros((T, H), np.float32)
    for e in range(E):
        s, c = int(starts[e]), int(counts[e])
        if c:
            out[order[s : s + c]] = res.results[e]["yT"][:, :c].T
    return out



# revision 3
# speedup vs baseline: 1.1302x; 1.1302x over previous
"""Expert-parallel MoE MLP (Llama4 text experts) for 8 Trainium2 NeuronCores.

Strategy: core e handles expert e. Tokens are grouped by expert on the host
(indices are sorted; argsort for robustness), padded to T_pad = max expert
count (rounded to 8), and each core runs a dense gated MLP over its block:
    y = (up * silu(gate)) @ W_d,   [gate | up] = x @ W_gu
computed transposed (weights stationary, tokens streaming):
    y^T = W_d^T @ h^T,  h^T = up^T * silu(W_gu^T @ x^T)
bf16 inputs, fp32 PSUM accumulation, fp32 output.

Perf notes vs the naive version:
  - All inputs land via 8 large contiguous DMAs (1-4 MB, 8-32 KB per-partition
    lines) instead of 128 small strided ones; the first matmul's deps (x block0
    + first weight pairs) are the first two DMAs issued.
  - Gate/up weight columns are pair-interleaved on the host so weights are
    consumed in DMA arrival order.
  - T_pad is the max expert count rounded to 8 (not 128), split into blocks
    (512, r, r) so there is no tiny tail block.
  - ~48 dummy matmuls on zeroed SBUF run during the initial DMA wait to lift
    the PE HAM clock gate before the real matmul stream begins.
"""

import numpy as np
import ml_dtypes

_BF16 = ml_dtypes.bfloat16
_NC = 8  # cores

_nc_cache: dict = {}
last_run = None  # BassKernelResults of the most recent kernel() call


def _build(T_pad: int, H: int, F: int):
    import concourse.bacc as bacc
    import concourse.mybir as mybir
    from concourse.tile import TileContext

    nc = bacc.Bacc()
    KB1 = H // 128        # contraction chunks for x @ W_gu (8)
    NPAIR = F // 128      # gate/up pairs (16)
    KB2 = F // 128        # contraction chunks for h @ W_d (16)
    NH = H // 128         # output tiles of y (8)
    T_r = T_pad - 512     # tokens in blocks 1+2
    nb1 = T_r // 2
    blocks = [(0, 512), (512, nb1), (512 + nb1, T_r - nb1)]

    bf16 = mybir.dt.bfloat16
    f32 = mybir.dt.float32

    # DRAM inputs: host-packed so every DMA is a whole-tensor contiguous copy.
    xb0 = nc.dram_tensor("xb0", [128, KB1 * 512], bf16, kind="ExternalInput")
    xb12 = nc.dram_tensor("xb12", [128, KB1 * T_r], bf16, kind="ExternalInput")
    whd = nc.dram_tensor("whd", [128, KB1 * 512], bf16, kind="ExternalInput")
    whd2 = nc.dram_tensor("whd2", [128, KB1 * 512], bf16, kind="ExternalInput")
    wgg = [
        nc.dram_tensor(f"wgg{g}", [128, KB1 * 1024], bf16, kind="ExternalInput")
        for g in range(3)
    ]
    wdp = nc.dram_tensor("wdp", [128, KB2 * 1024], bf16, kind="ExternalInput")
    yT = nc.dram_tensor("yT", [128, NH * T_pad], f32, kind="ExternalOutput")

    def gu_lhsT(p, k, up):
        # lhsT [128,128] for pair p (gate if not up), contraction chunk k
        off = 128 if up else 0
        if p < 2:
            return whd_sb[:, k * 512 + 256 * p + off : k * 512 + 256 * p + off + 128]
        if p < 4:
            q = p - 2
            return whd2_sb[:, k * 512 + 256 * q + off : k * 512 + 256 * q + off + 128]
        g, q = (p - 4) // 4, (p - 4) % 4
        return wg_sb[g][:, k * 1024 + 256 * q + off : k * 1024 + 256 * q + off + 128]

    def x_rhs(b, k):
        t0, nb = blocks[b]
        if b == 0:
            return x0_sb[:, k * 512 : k * 512 + nb]
        o = t0 - 512
        return x12_sb[:, k * T_r + o : k * T_r + o + nb]

    with TileContext(nc) as tc:
        with (
            tc.tile_pool(name="win", bufs=1) as win_p,
            tc.tile_pool(name="warm", bufs=1) as warm_p,
            tc.tile_pool(name="silu_p", bufs=3) as silu_p,
            tc.tile_pool(name="h_p", bufs=2) as h_p,
            tc.tile_pool(name="y_p", bufs=3) as y_p,
            tc.tile_pool(name="ps1", bufs=5, space="PSUM") as ps1_p,
            tc.tile_pool(name="ps2", bufs=3, space="PSUM") as ps2_p,
        ):
            # PE warm-up: dummy matmuls on zeroed SBUF while inputs stream in.
            wtile = warm_p.tile([128, 640], bf16, name="wtile", tag="wtile")
            nc.vector.memset(wtile[:], 0.0)
            for i in range(48):
                ps_w = ps2_p.tile([128, 512], f32, tag="ps2")
                nc.tensor.matmul(
                    out=ps_w[:, :128],
                    lhsT=wtile[:, 512:640],
                    rhs=wtile[:, 0:128],
                    start=True,
                    stop=True,
                )

            # Input DMAs, in consumption order. Each is one big contiguous copy.
            x0_sb = win_p.tile([128, KB1 * 512], bf16, name="x0", tag="x0")
            nc.sync.dma_start(out=x0_sb, in_=xb0[:, :])
            whd_sb = win_p.tile([128, KB1 * 512], bf16, name="whd", tag="whd")
            nc.sync.dma_start(out=whd_sb, in_=whd[:, :])
            whd2_sb = win_p.tile([128, KB1 * 512], bf16, name="whd2", tag="whd2")
            nc.sync.dma_start(out=whd2_sb, in_=whd2[:, :])
            wg_sb = []
            for g in range(3):
                t = win_p.tile([128, KB1 * 1024], bf16, name=f"wg{g}", tag=f"wg{g}")
                nc.sync.dma_start(out=t, in_=wgg[g][:, :])
                wg_sb.append(t)
            x12_sb = win_p.tile([128, KB1 * T_r], bf16, name="x12", tag="x12")
            nc.sync.dma_start(out=x12_sb, in_=xb12[:, :])
            wd_sb = win_p.tile([128, KB2 * 1024], bf16, name="wd", tag="wd")
            nc.sync.dma_start(out=wd_sb, in_=wdp[:, :])

            h_tiles = {}

            def gateup(b):
                t0, nb = blocks[b]
                for p in range(NPAIR):
                    ps_g = ps1_p.tile([128, 512], f32, tag="ps1")
                    for k in range(KB1):
                        nc.tensor.matmul(
                            out=ps_g[:, :nb],
                            lhsT=gu_lhsT(p, k, False),
                            rhs=x_rhs(b, k),
                            start=(k == 0),
                            stop=(k == KB1 - 1),
                        )
                    ps_u = ps1_p.tile([128, 512], f32, tag="ps1")
                    for k in range(KB1):
                        nc.tensor.matmul(
                            out=ps_u[:, :nb],
                            lhsT=gu_lhsT(p, k, True),
                            rhs=x_rhs(b, k),
                            start=(k == 0),
                            stop=(k == KB1 - 1),
                        )
                    st = silu_p.tile([128, 512], bf16, tag="silu")
                    nc.scalar.activation(
                        st[:, :nb], ps_g[:, :nb], mybir.ActivationFunctionType.Silu
                    )
                    ht = h_p.tile([128, 512], bf16, tag=f"h{p}")
                    nc.vector.tensor_mul(out=ht[:, :nb], in0=ps_u[:, :nb], in1=st[:, :nb])
                    h_tiles[(b, p)] = ht

            def down(b):
                t0, nb = blocks[b]
                for hh in range(NH):
                    ps_y = ps2_p.tile([128, 512], f32, tag="ps2")
                    for k in range(KB2):
                        nc.tensor.matmul(
                            out=ps_y[:, :nb],
                            lhsT=wd_sb[:, k * 1024 + 128 * hh : k * 1024 + 128 * hh + 128],
                            rhs=h_tiles[(b, k)][:, :nb],
                            start=(k == 0),
                            stop=(k == KB2 - 1),
                        )
                    yt = y_p.tile([128, 512], f32, tag="y")
                    nc.vector.tensor_copy(yt[:, :nb], ps_y[:, :nb])
                    nc.sync.dma_start(
                        out=yT[:, hh * T_pad + t0 : hh * T_pad + t0 + nb],
                        in_=yt[:, :nb],
                    )

            gateup(0)
            gateup(1)
            down(0)
            down(1)
            gateup(2)
            down(2)
    nc.compile()
    return nc


def kernel(hidden_states, local_expert_indices, gate_up_proj, down_proj):
    from concourse.bass_utils import run_bass_kernel_spmd

    x = np.asarray(hidden_states, dtype=np.float32)
    idx = np.asarray(local_expert_indices).astype(np.int64)
    wgu_all = np.asarray(gate_up_proj, dtype=np.float32)
    wd_all = np.asarray(down_proj, dtype=np.float32)

    T, H = x.shape
    E, _, F2 = wgu_all.shape
    F = F2 // 2
    assert E == _NC
    KB1 = H // 128

    order = np.argsort(idx, kind="stable")
    counts = np.bincount(idx, minlength=E)
    starts = np.concatenate([[0], np.cumsum(counts)])
    T_pad = max(528, int(-(-counts.max() // 8) * 8))
    T_r = T_pad - 512

    key = (T_pad, H, F)
    if key not in _nc_cache:
        _nc_cache[key] = _build(T_pad, H, F)
    nc = _nc_cache[key]

    x_sorted = x[order]
    in_maps = []
    for e in range(E):
        s, c = int(starts[e]), int(counts[e])
        xb = np.zeros((T_pad, H), np.float32)
        xb[:c] = x_sorted[s : s + c]
        xb = xb.astype(_BF16)
        # x chunk-major: [128 p][KB1 k][t]
        xb0 = np.ascontiguousarray(
            xb[:512].reshape(512, KB1, 128).transpose(2, 1, 0)
        ).reshape(128, KB1 * 512)
        xb12 = np.ascontiguousarray(
            xb[512:].reshape(T_r, KB1, 128).transpose(2, 1, 0)
        ).reshape(128, KB1 * T_r)
        # gate/up pair-interleaved columns: packed col block 256p = [gate_p | up_p]
        w = wgu_all[e].astype(_BF16)
        wg_ = w[:, :F].reshape(H, F // 128, 128)
        wu_ = w[:, F:].reshape(H, F // 128, 128)
        wp = np.empty((H, F // 128, 2, 128), _BF16)
        wp[:, :, 0] = wg_
        wp[:, :, 1] = wu_
        wp = wp.reshape(H, 2 * F)
        wpr = wp.reshape(KB1, 128, 2 * F).transpose(1, 0, 2)  # [p][k][c]
        whd = np.ascontiguousarray(wpr[:, :, 0:512]).reshape(128, KB1 * 512)
        whd2 = np.ascontiguousarray(wpr[:, :, 512:1024]).reshape(128, KB1 * 512)
        wggs = {
            f"wgg{g}": np.ascontiguousarray(
                wpr[:, :, 1024 + 1024 * g : 2048 + 1024 * g]
            ).reshape(128, KB1 * 1024)
            for g in range(3)
        }
        wdp = np.ascontiguousarray(
            wd_all[e].astype(_BF16).reshape(F // 128, 128, H).transpose(1, 0, 2)
        ).reshape(128, (F // 128) * H)
        in_maps.append(
            {"xb0": xb0, "xb12": xb12, "whd": whd, "whd2": whd2, "wdp": wdp, **wggs}
        )

    res = run_bass_kernel_spmd(nc, in_maps, core_ids=list(range(_NC)))
    global last_run
    last_run = res

    out = np.zeros((T, H), np.float32)
    for e in range(E):
        s, c = int(starts[e]), int(counts[e])
        if c:
            ye = np.asarray(res.results[e]["yT"]).reshape(128, H // 128, T_pad)
            y_pad = ye.transpose(2, 1, 0).reshape(T_pad, H)
            out[order[s : s + c]] = y_pad[:c]
    return out


# revision 10
# speedup vs baseline: 1.1391x; 1.0079x over previous
"""Expert-parallel MoE MLP (Llama4 text experts) for 8 Trainium2 NeuronCores.

Strategy: core e handles expert e. Tokens are grouped by expert on the host
(indices are sorted; argsort for robustness), padded to T_pad = max expert
count (rounded to 8), and each core runs a dense gated MLP over its block:
    y = (up * silu(gate)) @ W_d,   [gate | up] = x @ W_gu
computed transposed (weights stationary, tokens streaming):
    y^T = W_d^T @ h^T,  h^T = up^T * silu(W_gu^T @ x^T)
bf16 inputs, fp32 PSUM accumulation, fp32 output.

Perf notes vs the naive version:
  - All inputs land via 8 large contiguous DMAs (1-4 MB, 8-32 KB per-partition
    lines) instead of 128 small strided ones; the first matmul's deps (x block0
    + first weight pairs) are the first two DMAs issued.
  - Gate/up weight columns are pair-interleaved on the host so weights are
    consumed in DMA arrival order.
  - T_pad is the max expert count rounded to 8 (not 128), split into blocks
    (512, r, r) so there is no tiny tail block.
  - ~48 dummy matmuls on zeroed SBUF run during the initial DMA wait to lift
    the PE HAM clock gate before the real matmul stream begins.
"""

import numpy as np
import ml_dtypes

_BF16 = ml_dtypes.bfloat16
_NC = 8  # cores

_nc_cache: dict = {}
last_run = None  # BassKernelResults of the most recent kernel() call


def _build(T_pad: int, H: int, F: int):
    import concourse.bacc as bacc
    import concourse.mybir as mybir
    from concourse.tile import TileContext

    nc = bacc.Bacc()
    KB1 = H // 128        # contraction chunks for x @ W_gu (8)
    NPAIR = F // 128      # gate/up pairs (16)
    KB2 = F // 128        # contraction chunks for h @ W_d (16)
    NH = H // 128         # output tiles of y (8)
    T_r = T_pad - 512     # tokens in blocks 1+2
    nb1 = T_r // 2
    blocks = [(0, 512), (512, nb1), (512 + nb1, T_r - nb1)]

    bf16 = mybir.dt.bfloat16
    f32 = mybir.dt.float32

    # DRAM inputs: host-packed so every DMA is a whole-tensor contiguous copy.
    xb0 = nc.dram_tensor("xb0", [128, KB1 * 512], bf16, kind="ExternalInput")
    xb12 = nc.dram_tensor("xb12", [128, KB1 * T_r], bf16, kind="ExternalInput")
    wp0 = nc.dram_tensor("wp0", [128, KB1 * 256], bf16, kind="ExternalInput")
    wp1 = nc.dram_tensor("wp1", [128, KB1 * 256], bf16, kind="ExternalInput")
    whd2 = nc.dram_tensor("whd2", [128, KB1 * 512], bf16, kind="ExternalInput")
    wgg = [
        nc.dram_tensor(f"wgg{g}", [128, KB1 * 1024], bf16, kind="ExternalInput")
        for g in range(3)
    ]
    wdp = nc.dram_tensor("wdp", [128, KB2 * 1024], bf16, kind="ExternalInput")
    yT = nc.dram_tensor("yT", [128, NH * T_pad], f32, kind="ExternalOutput")

    def gu_lhsT(p, k, up):
        # lhsT [128,128] for pair p (gate if not up), contraction chunk k
        off = 128 if up else 0
        if p < 2:
            return wp_sb[p][:, k * 256 + off : k * 256 + off + 128]
        if p < 4:
            q = p - 2
            return whd2_sb[:, k * 512 + 256 * q + off : k * 512 + 256 * q + off + 128]
        g, q = (p - 4) // 4, (p - 4) % 4
        return wg_sb[g][:, k * 1024 + 256 * q + off : k * 1024 + 256 * q + off + 128]

    def x_rhs(b, k):
        t0, nb = blocks[b]
        if b == 0:
            return x0_sb[:, k * 512 : k * 512 + nb]
        o = t0 - 512
        return x12_sb[:, k * T_r + o : k * T_r + o + nb]

    with TileContext(nc) as tc:
        with (
            tc.tile_pool(name="win", bufs=1) as win_p,
            tc.tile_pool(name="warm", bufs=1) as warm_p,
            tc.tile_pool(name="silu_p", bufs=3) as silu_p,
            tc.tile_pool(name="h_p", bufs=2) as h_p,
            tc.tile_pool(name="y_p", bufs=3) as y_p,
            tc.tile_pool(name="ps1", bufs=5, space="PSUM") as ps1_p,
            tc.tile_pool(name="ps2", bufs=3, space="PSUM") as ps2_p,
        ):
            # PE warm-up: dummy matmuls on (uninitialized) SBUF while inputs
            # stream in. Results land in the ps2 rotation and are overwritten
            # by the first accumulation group (start=True) of each down tile.
            wtile = warm_p.tile([128, 256], bf16, name="wtile", tag="wtile")
            nc.vector.memset(wtile[:], 0.0)
            for i in range(64):
                ps_w = ps2_p.tile([128, 512], f32, tag="ps2")
                nc.tensor.matmul(
                    out=ps_w[:, :128],
                    lhsT=wtile[:, 128:256],
                    rhs=wtile[:, 0:128],
                    start=True,
                    stop=True,
                )

            # Input DMAs, in consumption order. Each is one big contiguous
            # copy. x goes on the scalar HWDGE ring, weights on the sync ring,
            # so the two streams start in parallel and weights keep strict
            # FIFO priority among themselves.
            x0_sb = win_p.tile([128, KB1 * 512], bf16, name="x0", tag="x0")
            nc.scalar.dma_start(out=x0_sb, in_=xb0[:, :])
            wp_sb = []
            for p in range(2):
                t = win_p.tile([128, KB1 * 256], bf16, name=f"wp{p}", tag=f"wp{p}")
                nc.sync.dma_start(out=t, in_=(wp0 if p == 0 else wp1)[:, :])
                wp_sb.append(t)
            whd2_sb = win_p.tile([128, KB1 * 512], bf16, name="whd2", tag="whd2")
            nc.sync.dma_start(out=whd2_sb, in_=whd2[:, :])
            x12_sb = win_p.tile([128, KB1 * T_r], bf16, name="x12", tag="x12")
            nc.scalar.dma_start(out=x12_sb, in_=xb12[:, :])
            wg_sb = []
            for g in range(3):
                t = win_p.tile([128, KB1 * 1024], bf16, name=f"wg{g}", tag=f"wg{g}")
                nc.sync.dma_start(out=t, in_=wgg[g][:, :])
                wg_sb.append(t)
            wd_sb = win_p.tile([128, KB2 * 1024], bf16, name="wd", tag="wd")
            nc.sync.dma_start(out=wd_sb, in_=wdp[:, :])

            h_tiles = {}

            def gateup(b):
                t0, nb = blocks[b]
                for p in range(NPAIR):
                    ps_g = ps1_p.tile([128, 512], f32, tag="ps1")
                    for k in range(KB1):
                        nc.tensor.matmul(
                            out=ps_g[:, :nb],
                            lhsT=gu_lhsT(p, k, False),
                            rhs=x_rhs(b, k),
                            start=(k == 0),
                            stop=(k == KB1 - 1),
                        )
                    ps_u = ps1_p.tile([128, 512], f32, tag="ps1")
                    for k in range(KB1):
                        nc.tensor.matmul(
                            out=ps_u[:, :nb],
                            lhsT=gu_lhsT(p, k, True),
                            rhs=x_rhs(b, k),
                            start=(k == 0),
                            stop=(k == KB1 - 1),
                        )
                    st = silu_p.tile([128, 512], bf16, tag="silu")
                    nc.scalar.activation(
                        st[:, :nb], ps_g[:, :nb], mybir.ActivationFunctionType.Silu
                    )
                    ht = h_p.tile([128, 512], bf16, tag=f"h{p}")
                    nc.vector.tensor_mul(out=ht[:, :nb], in0=ps_u[:, :nb], in1=st[:, :nb])
                    h_tiles[(b, p)] = ht

            def down(b):
                t0, nb = blocks[b]
                for hh in range(NH):
                    ps_y = ps2_p.tile([128, 512], f32, tag="ps2")
                    for k in range(KB2):
                        nc.tensor.matmul(
                            out=ps_y[:, :nb],
                            lhsT=wd_sb[:, k * 1024 + 128 * hh : k * 1024 + 128 * hh + 128],
                            rhs=h_tiles[(b, k)][:, :nb],
                            start=(k == 0),
                            stop=(k == KB2 - 1),
                        )
                    yt = y_p.tile([128, 512], f32, tag="y")
                    nc.vector.tensor_copy(yt[:, :nb], ps_y[:, :nb])
                    nc.sync.dma_start(
                        out=yT[:, hh * T_pad + t0 : hh * T_pad + t0 + nb],
                        in_=yt[:, :nb],
                    )

            gateup(0)
            gateup(1)
            down(0)
            down(1)
            gateup(2)
            down(2)
    nc.compile()
    return nc


def kernel(hidden_states, local_expert_indices, gate_up_proj, down_proj):
    from concourse.bass_utils import run_bass_kernel_spmd

    x = np.asarray(hidden_states, dtype=np.float32)
    idx = np.asarray(local_expert_indices).astype(np.int64)
    wgu_all = np.asarray(gate_up_proj, dtype=np.float32)
    wd_all = np.asarray(down_proj, dtype=np.float32)

    T, H = x.shape
    E, _, F2 = wgu_all.shape
    F = F2 // 2
    assert E == _NC
    KB1 = H // 128

    order = np.argsort(idx, kind="stable")
    counts = np.bincount(idx, minlength=E)
    starts = np.concatenate([[0], np.cumsum(counts)])
    T_pad = max(520, int(-(-counts.max() // 4) * 4))
    if (T_pad - 512) % 2:
        T_pad += 2
    T_r = T_pad - 512

    key = (T_pad, H, F)
    if key not in _nc_cache:
        _nc_cache[key] = _build(T_pad, H, F)
    nc = _nc_cache[key]

    x_sorted = x[order]
    in_maps = []
    for e in range(E):
        s, c = int(starts[e]), int(counts[e])
        xb = np.zeros((T_pad, H), np.float32)
        xb[:c] = x_sorted[s : s + c]
        xb = xb.astype(_BF16)
        # x chunk-major: [128 p][KB1 k][t]
        xb0 = np.ascontiguousarray(
            xb[:512].reshape(512, KB1, 128).transpose(2, 1, 0)
        ).reshape(128, KB1 * 512)
        xb12 = np.ascontiguousarray(
            xb[512:].reshape(T_r, KB1, 128).transpose(2, 1, 0)
        ).reshape(128, KB1 * T_r)
        # gate/up pair-interleaved columns: packed col block 256p = [gate_p | up_p]
        w = wgu_all[e].astype(_BF16)
        wg_ = w[:, :F].reshape(H, F // 128, 128)
        wu_ = w[:, F:].reshape(H, F // 128, 128)
        wp = np.empty((H, F // 128, 2, 128), _BF16)
        wp[:, :, 0] = wg_
        wp[:, :, 1] = wu_
        wp = wp.reshape(H, 2 * F)
        wpr = wp.reshape(KB1, 128, 2 * F).transpose(1, 0, 2)  # [p][k][c]
        wp0 = np.ascontiguousarray(wpr[:, :, 0:256]).reshape(128, KB1 * 256)
        wp1 = np.ascontiguousarray(wpr[:, :, 256:512]).reshape(128, KB1 * 256)
        whd2 = np.ascontiguousarray(wpr[:, :, 512:1024]).reshape(128, KB1 * 512)
        wggs = {
            f"wgg{g}": np.ascontiguousarray(
                wpr[:, :, 1024 + 1024 * g : 2048 + 1024 * g]
            ).reshape(128, KB1 * 1024)
            for g in range(3)
        }
        wdp = np.ascontiguousarray(
            wd_all[e].astype(_BF16).reshape(F // 128, 128, H).transpose(1, 0, 2)
        ).reshape(128, (F // 128) * H)
        in_maps.append(
            {"xb0": xb0, "xb12": xb12, "wp0": wp0, "wp1": wp1, "whd2": whd2, "wdp": wdp, **wggs}
        )

    res = run_bass_kernel_spmd(nc, in_maps, core_ids=list(range(_NC)))
    global last_run
    last_run = res

    out = np.zeros((T, H), np.float32)
    for e in range(E):
        s, c = int(starts[e]), int(counts[e])
        if c:
            ye = np.asarray(res.results[e]["yT"]).reshape(128, H // 128, T_pad)
            y_pad = ye.transpose(2, 1, 0).reshape(T_pad, H)
            out[order[s : s + c]] = y_pad[:c]
    return out
